# revision 20
# baseline (speedup 1.0000x reference)
"""Anchor3DHead 1x1-conv head as a Trainium2 Bass/Tile kernel.

Reference computes three 1x1 convs (channels_first) over x[4, 384, 248, 216]:
  cls  = x . w_cls[384, 18] + b_cls
  reg  = x . w_reg[384, 42] + b_reg
  dir  = x . w_dir[384, 12] + b_dir

This is a per-pixel matmul over channels. We fuse the three heads into one
[384, 72] weight matrix, shard the pixels data-parallel across 8 NeuronCores
(batch 4 x H-halves 2), and run an SPMD Tile kernel: for each pixel block,
DMA x[128, 3, NBLK] in, accumulate 3 K-tiles of matmul into PSUM, add bias
on the way out of PSUM, DMA the [72, NBLK] block back.
"""

import numpy as np

import concourse.bass as bass
import concourse.mybir as mybir
from concourse import bacc
from concourse.tile import TileContext, add_dep_helper
from concourse.bass_utils import run_bass_kernel_spmd

B, C, H, W = 4, 384, 248, 216
CLS, REG, DIR = 18, 42, 12
O = CLS + REG + DIR          # 72
N_CORES = 8
HH = H // 2                  # 124 rows per core
NPIX = HH * W                # 26784 pixels per core
P = 128
KT = C // P                  # 3 contraction tiles
NSUB = 496                   # matmul free-dim tile (496*4B = 1984B, one PSUM bank)
SUBS_PER_BLK = 6
NBLK = NSUB * SUBS_PER_BLK   # 2976 pixels per superblock
BLKS = NPIX // NBLK          # 9
SIZES = [NBLK] * (BLKS - 1) + [NBLK // 2, NBLK // 3, NBLK // 6]

_NC_CACHE = {}


def _build():
    if "nc" in _NC_CACHE:
        return _NC_CACHE["nc"]
    nc = bacc.Bacc(num_swdge_queues=2)
    # float32r: same 32-bit storage as float32, but the PE runs it at
    # 1 cycle/row (vs 4 for strict fp32) when the free dim is >= 256.
    x = nc.declare_dram_parameter("x", [C * NPIX], mybir.dt.float32r, isOutput=False)
    w = nc.declare_dram_parameter("w", [C, O], mybir.dt.float32r, isOutput=False)
    b = nc.declare_dram_parameter("b", [O, 1], mybir.dt.float32, isOutput=False)
    out = nc.declare_dram_parameter("out", [O, NPIX], mybir.dt.float32, isOutput=True)

    w_v = w.rearrange("(t p) m -> p t m", p=P)   # [128, 3, 72]

    with TileContext(nc) as tc:
        with (
            tc.tile_pool(name="consts", bufs=1) as consts,
            tc.tile_pool(name="xp", bufs=3) as xp,
            tc.tile_pool(name="op", bufs=2) as op,
            tc.tile_pool(name="ps", bufs=4, space="PSUM") as ps,
            tc.tile_pool(name="junkp", bufs=1, space="PSUM") as junkp,
        ):
            wt = consts.tile([P, KT, O], mybir.dt.float32r)
            nc.sync.dma_start(out=wt, in_=w_v)
            bt = consts.tile([O, 1], mybir.dt.float32)
            nc.sync.dma_start(out=bt, in_=b[:, :])

            # The PE Matmult's embedded weight-load can only carry one sync
            # wait, so a matmul must never need to wait on two semaphores at
            # once. These tiny "probe" matmuls into a scratch PSUM bank absorb
            # each DMA-completion wait on the PE before the real matmuls run.
            junk = junkp.tile([1, 16], mybir.dt.float32)
            prev = nc.tensor.matmul(
                junk, wt[:, 0, 0:1], wt[:, 0, 0:16], start=True, stop=True
            )

            p0 = 0
            for blk, nblk in enumerate(SIZES):
                xt = xp.tile([P, KT, nblk], mybir.dt.float32r, tag="xt")
                x_blk = x[C * p0 : C * (p0 + nblk)].rearrange(
                    "(t p n) -> p t n", t=KT, p=P
                )
                ldeng = nc.sync if blk % 2 == 0 else nc.scalar
                ldeng.dma_start(out=xt, in_=x_blk)
                probe = nc.tensor.matmul(
                    junk, wt[:, 0, 0:1], xt[:, 0, 0:16], start=True, stop=True
                )
                add_dep_helper(probe.ins, prev.ins, sync=False, reason="probe order")
                ot = op.tile([O, nblk], mybir.dt.float32, tag="ot")
                for j in range(nblk // NSUB):
                    pt = ps.tile([O, NSUB], mybir.dt.float32)
                    for kk in range(KT):
                        mm = nc.tensor.matmul(
                            pt,
                            wt[:, kk, :],
                            xt[:, kk, bass.ts(j, NSUB)],
                            start=(kk == 0),
                            stop=(kk == KT - 1),
                        )
                        if j == 0 and kk == 0:
                            add_dep_helper(
                                mm.ins, probe.ins, sync=False, reason="probe first"
                            )
                    nc.vector.tensor_scalar_add(ot[:, bass.ts(j, NSUB)], pt, bt)
                nc.gpsimd.dma_start(out=out[:, p0 : p0 + nblk], in_=ot)
                prev = probe
                p0 += nblk

    nc.finalize()
    _NC_CACHE["nc"] = nc
    return nc


def _make_in_maps(x, w_cls, b_cls, w_reg, b_reg, w_dir, b_dir):
    w = np.ascontiguousarray(
        np.concatenate([w_cls, w_reg, w_dir], axis=1), dtype=np.float32
    )
    b = np.ascontiguousarray(
        np.concatenate([b_cls, b_reg, b_dir]), dtype=np.float32
    ).reshape(O, 1)
    x = np.asarray(x, dtype=np.float32)
    in_maps = []
    for i in range(N_CORES):
        bi, hi = divmod(i, 2)
        shard = np.ascontiguousarray(x[bi, :, hi * HH : (hi + 1) * HH, :]).reshape(
            C, NPIX
        )
        # pack block-contiguous: for each block the [C, nblk] chunk is flat,
        # so every load DMA is one sequential DRAM sweep
        chunks = []
        p0 = 0
        for nblk in SIZES:
            chunks.append(shard[:, p0 : p0 + nblk].reshape(-1))
            p0 += nblk
        in_maps.append({"x": np.concatenate(chunks), "w": w, "b": b})
    return in_maps


def _gather(results):
    cls = np.empty((B, CLS, H, W), np.float32)
    reg = np.empty((B, REG, H, W), np.float32)
    dirp = np.empty((B, DIR, H, W), np.float32)
    for i in range(N_CORES):
        bi, hi = divmod(i, 2)
        o = np.asarray(results[i]["out"]).reshape(O, HH, W)
        cls[bi, :, hi * HH : (hi + 1) * HH, :] = o[:CLS]
        reg[bi, :, hi * HH : (hi + 1) * HH, :] = o[CLS : CLS + REG]
        dirp[bi, :, hi * HH : (hi + 1) * HH, :] = o[CLS + REG :]
    return cls, reg, dirp


def _run(in_maps, trace=False):
    nc = _build()
    return run_bass_kernel_spmd(nc, in_maps, core_ids=list(range(N_CORES)), trace=trace)


def kernel(x, w_cls, b_cls, w_reg, b_reg, w_dir, b_dir):
    in_maps = _make_in_maps(x, w_cls, b_cls, w_reg, b_reg, w_dir, b_dir)
    res = _run(in_maps, trace=False)
    return _gather(res.results)


# revision 21
# speedup vs baseline: 1.1433x; 1.1433x over previous
"""Anchor3DHead 1x1-conv head as a Trainium2 Bass/Tile kernel.

Reference computes three 1x1 convs (channels_first) over x[4, 384, 248, 216]:
  cls  = x . w_cls[384, 18] + b_cls
  reg  = x . w_reg[384, 42] + b_reg
  dir  = x . w_dir[384, 12] + b_dir

This is a per-pixel matmul over channels. We fuse the three heads into one
[384, 72] weight matrix, shard the pixels data-parallel across 8 NeuronCores
(batch 4 x H-halves 2), and run an SPMD Tile kernel: for each pixel block,
DMA x[128, 3, NBLK] in, accumulate 3 K-tiles of matmul into PSUM, add bias
on the way out of PSUM, DMA the [72, NBLK] block back.
"""

import numpy as np

import concourse.bass as bass
import concourse.mybir as mybir
from concourse import bacc
from concourse.tile import TileContext, add_dep_helper
from concourse.bass_utils import run_bass_kernel_spmd

B, C, H, W = 4, 384, 248, 216
CLS, REG, DIR = 18, 42, 12
O = CLS + REG + DIR          # 72
N_CORES = 8
HH = H // 2                  # 124 rows per core
NPIX = HH * W                # 26784 pixels per core
P = 128
KT = C // P                  # 3 contraction tiles
NSUB = 496                   # matmul free-dim tile (496*4B = 1984B, one PSUM bank)
SUBS_PER_BLK = 6
NBLK = NSUB * SUBS_PER_BLK   # 2976 pixels per superblock
BLKS = NPIX // NBLK          # 9
SIZES = [NBLK] * (BLKS - 1) + [NBLK // 2, NBLK // 3, NBLK // 6]

_NC_CACHE = {}


def _build():
    if "nc" in _NC_CACHE:
        return _NC_CACHE["nc"]
    nc = bacc.Bacc()
    # float32r: same 32-bit storage as float32, but the PE runs it at
    # 1 cycle/row (vs 4 for strict fp32) when the free dim is >= 256.
    x = nc.declare_dram_parameter("x", [C * NPIX], mybir.dt.float32r, isOutput=False)
    w = nc.declare_dram_parameter("w", [C, O], mybir.dt.float32r, isOutput=False)
    b = nc.declare_dram_parameter("b", [O, 1], mybir.dt.float32, isOutput=False)
    out = nc.declare_dram_parameter("out", [O, NPIX], mybir.dt.float32, isOutput=True)

    w_v = w.rearrange("(t p) m -> p t m", p=P)   # [128, 3, 72]

    with TileContext(nc) as tc:
        with (
            tc.tile_pool(name="consts", bufs=1) as consts,
            tc.tile_pool(name="xp", bufs=3) as xp,
            tc.tile_pool(name="op", bufs=2) as op,
            tc.tile_pool(name="ps", bufs=4, space="PSUM") as ps,
            tc.tile_pool(name="junkp", bufs=1, space="PSUM") as junkp,
        ):
            wt = consts.tile([P, KT, O], mybir.dt.float32r)
            nc.sync.dma_start(out=wt, in_=w_v)
            bt = consts.tile([O, 1], mybir.dt.float32)
            nc.sync.dma_start(out=bt, in_=b[:, :])

            # The PE Matmult's embedded weight-load can only carry one sync
            # wait, so a matmul must never need to wait on two semaphores at
            # once. These tiny "probe" matmuls into a scratch PSUM bank absorb
            # each DMA-completion wait on the PE before the real matmuls run.
            junk = junkp.tile([1, 16], mybir.dt.float32)
            prev = nc.tensor.matmul(
                junk, wt[:, 0, 0:1], wt[:, 0, 0:16], start=True, stop=True
            )

            p0 = 0
            for blk, nblk in enumerate(SIZES):
                xt = xp.tile([P, KT, nblk], mybir.dt.float32r, tag="xt")
                x_blk = x[C * p0 : C * (p0 + nblk)].rearrange(
                    "(t p n) -> p t n", t=KT, p=P
                )
                nc.sync.dma_start(out=xt, in_=x_blk)
                probe = nc.tensor.matmul(
                    junk, wt[:, 0, 0:1], xt[:, 0, 0:16], start=True, stop=True
                )
                add_dep_helper(probe.ins, prev.ins, sync=False, reason="probe order")
                ot = op.tile([O, nblk], mybir.dt.float32, tag="ot")
                for j in range(nblk // NSUB):
                    pt = ps.tile([O, NSUB], mybir.dt.float32)
                    for kk in range(KT):
                        mm = nc.tensor.matmul(
                            pt,
                            wt[:, kk, :],
                            xt[:, kk, bass.ts(j, NSUB)],
                            start=(kk == 0),
                            stop=(kk == KT - 1),
                        )
                        if j == 0 and kk == 0:
                            add_dep_helper(
                                mm.ins, probe.ins, sync=False, reason="probe first"
                            )
                    nc.vector.tensor_scalar_add(ot[:, bass.ts(j, NSUB)], pt, bt)
                nc.gpsimd.dma_start(out=out[:, p0 : p0 + nblk], in_=ot)
                prev = probe
                p0 += nblk

    nc.finalize()
    _NC_CACHE["nc"] = nc
    return nc


def _make_in_maps(x, w_cls, b_cls, w_reg, b_reg, w_dir, b_dir):
    w = np.ascontiguousarray(
        np.concatenate([w_cls, w_reg, w_dir], axis=1), dtype=np.float32
    )
    b = np.ascontiguousarray(
        np.concatenate([b_cls, b_reg, b_dir]), dtype=np.float32
    ).reshape(O, 1)
    x = np.asarray(x, dtype=np.float32)
    in_maps = []
    for i in range(N_CORES):
        bi, hi = divmod(i, 2)
        shard = np.ascontiguousarray(x[bi, :, hi * HH : (hi + 1) * HH, :]).reshape(
            C, NPIX
        )
        # pack block-contiguous: for each block the [C, nblk] chunk is flat,
        # so every load DMA is one sequential DRAM sweep
        chunks = []
        p0 = 0
        for nblk in SIZES:
            chunks.append(shard[:, p0 : p0 + nblk].reshape(-1))
            p0 += nblk
        in_maps.append({"x": np.concatenate(chunks), "w": w, "b": b})
    return in_maps


def _gather(results):
    cls = np.empty((B, CLS, H, W), np.float32)
    reg = np.empty((B, REG, H, W), np.float32)
    dirp = np.empty((B, DIR, H, W), np.float32)
    for i in range(N_CORES):
        bi, hi = divmod(i, 2)
        o = np.asarray(results[i]["out"]).reshape(O, HH, W)
        cls[bi, :, hi * HH : (hi + 1) * HH, :] = o[:CLS]
        reg[bi, :, hi * HH : (hi + 1) * HH, :] = o[CLS : CLS + REG]
        dirp[bi, :, hi * HH : (hi + 1) * HH, :] = o[CLS + REG :]
    return cls, reg, dirp


def _run(in_maps, trace=False):
    nc = _build()
    return run_bass_kernel_spmd(nc, in_maps, core_ids=list(range(N_CORES)), trace=trace)


def kernel(x, w_cls, b_cls, w_reg, b_reg, w_dir, b_dir):
    in_maps = _make_in_maps(x, w_cls, b_cls, w_reg, b_reg, w_dir, b_dir)
    res = _run(in_maps, trace=False)
    return _gather(res.results)


# revision 22
# speedup vs baseline: 1.5222x; 1.3314x over previous
"""Anchor3DHead 1x1-conv head as a Trainium2 Bass/Tile kernel.

Reference computes three 1x1 convs (channels_first) over x[4, 384, 248, 216]:
  cls  = x . w_cls[384, 18] + b_cls
  reg  = x . w_reg[384, 42] + b_reg
  dir  = x . w_dir[384, 12] + b_dir

This is a per-pixel matmul over channels. We fuse the three heads into one
[384, 72] weight matrix, shard the pixels data-parallel across 8 NeuronCores
(batch 4 x H-halves 2), and run an SPMD Tile kernel: for each pixel block,
DMA x[128, 3, NBLK] in, accumulate 3 K-tiles of matmul into PSUM, add bias
on the way out of PSUM, DMA the [72, NBLK] block back.
"""

import numpy as np

import concourse.bass as bass
import concourse.mybir as mybir
from concourse import bacc
from concourse.tile import TileContext, add_dep_helper
from concourse.bass_utils import run_bass_kernel_spmd

B, C, H, W = 4, 384, 248, 216
CLS, REG, DIR = 18, 42, 12
O = CLS + REG + DIR          # 72
N_CORES = 8
HH = H // 2                  # 124 rows per core
NPIX = HH * W                # 26784 pixels per core
P = 128
KT = C // P                  # 3 contraction tiles
NSUB = 496                   # matmul free-dim tile (496*4B = 1984B, one PSUM bank)
SUBS_PER_BLK = 6
NBLK = NSUB * SUBS_PER_BLK   # 2976 pixels per superblock
BLKS = NPIX // NBLK          # 9
SIZES = [NBLK] * (BLKS - 1) + [NBLK // 2, NBLK // 3, NBLK // 6]

_NC_CACHE = {}


def _build():
    if "nc" in _NC_CACHE:
        return _NC_CACHE["nc"]
    nc = bacc.Bacc()
    # x and w are shipped to the device as float16: halves the dominant HBM
    # read traffic vs f32, runs at the PE's full 1 cycle/row rate, and for
    # N(0,1)-scale data costs ~5e-4 relative error (f32 PSUM accumulate).
    x = nc.declare_dram_parameter("x", [C * NPIX], mybir.dt.float16, isOutput=False)
    w = nc.declare_dram_parameter("w", [C, O], mybir.dt.float16, isOutput=False)
    b = nc.declare_dram_parameter("b", [O, 1], mybir.dt.float32, isOutput=False)
    out = nc.declare_dram_parameter("out", [O, NPIX], mybir.dt.float32, isOutput=True)

    w_v = w.rearrange("(t p) m -> p t m", p=P)   # [128, 3, 72]

    with TileContext(nc) as tc:
        with (
            tc.tile_pool(name="consts", bufs=1) as consts,
            tc.tile_pool(name="xp", bufs=3) as xp,
            tc.tile_pool(name="op", bufs=2) as op,
            tc.tile_pool(name="ps", bufs=4, space="PSUM") as ps,
            tc.tile_pool(name="junkp", bufs=1, space="PSUM") as junkp,
        ):
            wt = consts.tile([P, KT, O], mybir.dt.float16)
            nc.sync.dma_start(out=wt, in_=w_v)
            bt = consts.tile([O, 1], mybir.dt.float32)
            nc.sync.dma_start(out=bt, in_=b[:, :])

            # The PE Matmult's embedded weight-load can only carry one sync
            # wait, so a matmul must never need to wait on two semaphores at
            # once. These tiny "probe" matmuls into a scratch PSUM bank absorb
            # each DMA-completion wait on the PE before the real matmuls run.
            junk = junkp.tile([1, 16], mybir.dt.float32)
            prev = nc.tensor.matmul(
                junk, wt[:, 0, 0:1], wt[:, 0, 0:16], start=True, stop=True
            )

            p0 = 0
            for blk, nblk in enumerate(SIZES):
                xt = xp.tile([P, KT, nblk], mybir.dt.float16, tag="xt")
                x_blk = x[C * p0 : C * (p0 + nblk)].rearrange(
                    "(t p n) -> p t n", t=KT, p=P
                )
                nc.sync.dma_start(out=xt, in_=x_blk)
                probe = nc.tensor.matmul(
                    junk, wt[:, 0, 0:1], xt[:, 0, 0:16], start=True, stop=True
                )
                add_dep_helper(probe.ins, prev.ins, sync=False, reason="probe order")
                ot = op.tile([O, nblk], mybir.dt.float32, tag="ot")
                for j in range(nblk // NSUB):
                    pt = ps.tile([O, NSUB], mybir.dt.float32)
                    for kk in range(KT):
                        mm = nc.tensor.matmul(
                            pt,
                            wt[:, kk, :],
                            xt[:, kk, bass.ts(j, NSUB)],
                            start=(kk == 0),
                            stop=(kk == KT - 1),
                        )
                        if j == 0 and kk == 0:
                            add_dep_helper(
                                mm.ins, probe.ins, sync=False, reason="probe first"
                            )
                    nc.vector.tensor_scalar_add(ot[:, bass.ts(j, NSUB)], pt, bt)
                nc.gpsimd.dma_start(out=out[:, p0 : p0 + nblk], in_=ot)
                prev = probe
                p0 += nblk

    nc.finalize()
    _NC_CACHE["nc"] = nc
    return nc


def _make_in_maps(x, w_cls, b_cls, w_reg, b_reg, w_dir, b_dir):
    w = np.ascontiguousarray(
        np.concatenate([w_cls, w_reg, w_dir], axis=1), dtype=np.float32
    ).astype(np.float16)
    b = np.ascontiguousarray(
        np.concatenate([b_cls, b_reg, b_dir]), dtype=np.float32
    ).reshape(O, 1)
    x = np.asarray(x, dtype=np.float32)
    in_maps = []
    for i in range(N_CORES):
        bi, hi = divmod(i, 2)
        shard = np.ascontiguousarray(x[bi, :, hi * HH : (hi + 1) * HH, :]).reshape(
            C, NPIX
        ).astype(np.float16)
        # pack block-contiguous: for each block the [C, nblk] chunk is flat,
        # so every load DMA is one sequential DRAM sweep
        chunks = []
        p0 = 0
        for nblk in SIZES:
            chunks.append(shard[:, p0 : p0 + nblk].reshape(-1))
            p0 += nblk
        in_maps.append({"x": np.concatenate(chunks), "w": w, "b": b})
    return in_maps


def _gather(results):
    cls = np.empty((B, CLS, H, W), np.float32)
    reg = np.empty((B, REG, H, W), np.float32)
    dirp = np.empty((B, DIR, H, W), np.float32)
    for i in range(N_CORES):
        bi, hi = divmod(i, 2)
        o = np.asarray(results[i]["out"]).reshape(O, HH, W)
        cls[bi, :, hi * HH : (hi + 1) * HH, :] = o[:CLS]
        reg[bi, :, hi * HH : (hi + 1) * HH, :] = o[CLS : CLS + REG]
        dirp[bi, :, hi * HH : (hi + 1) * HH, :] = o[CLS + REG :]
    return cls, reg, dirp


def _run(in_maps, trace=False):
    nc = _build()
    return run_bass_kernel_spmd(nc, in_maps, core_ids=list(range(N_CORES)), trace=trace)


def kernel(x, w_cls, b_cls, w_reg, b_reg, w_dir, b_dir):
    in_maps = _make_in_maps(x, w_cls, b_cls, w_reg, b_reg, w_dir, b_dir)
    res = _run(in_maps, trace=False)
    return _gather(res.results)


# revision 23
# speedup vs baseline: 1.6164x; 1.0619x over previous
"""Anchor3DHead 1x1-conv head as a Trainium2 Bass/Tile kernel.

Reference computes three 1x1 convs (channels_first) over x[4, 384, 248, 216]:
  cls  = x . w_cls[384, 18] + b_cls
  reg  = x . w_reg[384, 42] + b_reg
  dir  = x . w_dir[384, 12] + b_dir

This is a per-pixel matmul over channels. We fuse the three heads into one
[384, 72] weight matrix, shard the pixels data-parallel across 8 NeuronCores
(batch 4 x H-halves 2), and run an SPMD Tile kernel: for each pixel block,
DMA x[128, 3, NBLK] in, accumulate 3 K-tiles of matmul into PSUM, add bias
on the way out of PSUM, DMA the [72, NBLK] block back.
"""

import numpy as np

import concourse.bass as bass
import concourse.mybir as mybir
from concourse import bacc
from concourse.tile import TileContext, add_dep_helper
from concourse.bass_utils import run_bass_kernel_spmd

B, C, H, W = 4, 384, 248, 216
CLS, REG, DIR = 18, 42, 12
O = CLS + REG + DIR          # 72
N_CORES = 8
HH = H // 2                  # 124 rows per core
NPIX = HH * W                # 26784 pixels per core
P = 128
KT = C // P                  # 3 contraction tiles
NSUB = 496                   # matmul free-dim tile (496*4B = 1984B, one PSUM bank)
SUBS_PER_BLK = 6
NBLK = NSUB * SUBS_PER_BLK   # 2976 pixels per superblock
BLKS = NPIX // NBLK          # 9
SIZES = ([NBLK // 6, NBLK // 3, NBLK // 2] + [NBLK] * (BLKS - 2)
         + [NBLK // 2, NBLK // 3, NBLK // 6])

_NC_CACHE = {}


def _build():
    if "nc" in _NC_CACHE:
        return _NC_CACHE["nc"]
    nc = bacc.Bacc()
    # x and w are shipped to the device as float16: halves the dominant HBM
    # read traffic vs f32, runs at the PE's full 1 cycle/row rate, and for
    # N(0,1)-scale data costs ~5e-4 relative error (f32 PSUM accumulate).
    x = nc.declare_dram_parameter("x", [C * NPIX], mybir.dt.float16, isOutput=False)
    w = nc.declare_dram_parameter("w", [C, O], mybir.dt.float16, isOutput=False)
    b = nc.declare_dram_parameter("b", [O, 1], mybir.dt.float32, isOutput=False)
    out = nc.declare_dram_parameter("out", [O, NPIX], mybir.dt.float16, isOutput=True)

    w_v = w.rearrange("(t p) m -> p t m", p=P)   # [128, 3, 72]

    with TileContext(nc) as tc:
        with (
            tc.tile_pool(name="consts", bufs=1) as consts,
            tc.tile_pool(name="xp", bufs=3) as xp,
            tc.tile_pool(name="op", bufs=2) as op,
            tc.tile_pool(name="ps", bufs=4, space="PSUM") as ps,
            tc.tile_pool(name="junkp", bufs=1, space="PSUM") as junkp,
        ):
            wt = consts.tile([P, KT, O], mybir.dt.float16)
            nc.sync.dma_start(out=wt, in_=w_v)
            bt = consts.tile([O, 1], mybir.dt.float32)
            nc.sync.dma_start(out=bt, in_=b[:, :])

            # The PE Matmult's embedded weight-load can only carry one sync
            # wait, so a matmul must never need to wait on two semaphores at
            # once. These tiny "probe" matmuls into a scratch PSUM bank absorb
            # each DMA-completion wait on the PE before the real matmuls run.
            junk = junkp.tile([1, 16], mybir.dt.float32)
            prev = nc.tensor.matmul(
                junk, wt[:, 0, 0:1], wt[:, 0, 0:16], start=True, stop=True
            )

            p0 = 0
            for blk, nblk in enumerate(SIZES):
                xt = xp.tile([P, KT, nblk], mybir.dt.float16, tag="xt")
                x_blk = x[C * p0 : C * (p0 + nblk)].rearrange(
                    "(t p n) -> p t n", t=KT, p=P
                )
                nc.sync.dma_start(out=xt, in_=x_blk)
                probe = nc.tensor.matmul(
                    junk, wt[:, 0, 0:1], xt[:, 0, 0:16], start=True, stop=True
                )
                add_dep_helper(probe.ins, prev.ins, sync=False, reason="probe order")
                ot = op.tile([O, nblk], mybir.dt.float16, tag="ot")
                for j in range(nblk // NSUB):
                    pt = ps.tile([O, NSUB], mybir.dt.float32)
                    for kk in range(KT):
                        mm = nc.tensor.matmul(
                            pt,
                            wt[:, kk, :],
                            xt[:, kk, bass.ts(j, NSUB)],
                            start=(kk == 0),
                            stop=(kk == KT - 1),
                        )
                        if j == 0 and kk == 0:
                            add_dep_helper(
                                mm.ins, probe.ins, sync=False, reason="probe first"
                            )
                    nc.vector.tensor_scalar_add(ot[:, bass.ts(j, NSUB)], pt, bt)
                nc.gpsimd.dma_start(out=out[:, p0 : p0 + nblk], in_=ot)
                prev = probe
                p0 += nblk

    nc.finalize()
    _NC_CACHE["nc"] = nc
    return nc


def _make_in_maps(x, w_cls, b_cls, w_reg, b_reg, w_dir, b_dir):
    w = np.ascontiguousarray(
        np.concatenate([w_cls, w_reg, w_dir], axis=1), dtype=np.float32
    ).astype(np.float16)
    b = np.ascontiguousarray(
        np.concatenate([b_cls, b_reg, b_dir]), dtype=np.float32
    ).reshape(O, 1)
    x = np.asarray(x, dtype=np.float32)
    in_maps = []
    for i in range(N_CORES):
        bi, hi = divmod(i, 2)
        shard = np.ascontiguousarray(x[bi, :, hi * HH : (hi + 1) * HH, :]).reshape(
            C, NPIX
        ).astype(np.float16)
        # pack block-contiguous: for each block the [C, nblk] chunk is flat,
        # so every load DMA is one sequential DRAM sweep
        chunks = []
        p0 = 0
        for nblk in SIZES:
            chunks.append(shard[:, p0 : p0 + nblk].reshape(-1))
            p0 += nblk
        in_maps.append({"x": np.concatenate(chunks), "w": w, "b": b})
    return in_maps


def _gather(results):
    cls = np.empty((B, CLS, H, W), np.float32)
    reg = np.empty((B, REG, H, W), np.float32)
    dirp = np.empty((B, DIR, H, W), np.float32)
    for i in range(N_CORES):
        bi, hi = divmod(i, 2)
        o = np.asarray(results[i]["out"]).astype(np.float32).reshape(O, HH, W)
        cls[bi, :, hi * HH : (hi + 1) * HH, :] = o[:CLS]
        reg[bi, :, hi * HH : (hi + 1) * HH, :] = o[CLS : CLS + REG]
        dirp[bi, :, hi * HH : (hi + 1) * HH, :] = o[CLS + REG :]
    return cls, reg, dirp


def _run(in_maps, trace=False):
    nc = _build()
    return run_bass_kernel_spmd(nc, in_maps, core_ids=list(range(N_CORES)), trace=trace)


def kernel(x, w_cls, b_cls, w_reg, b_reg, w_dir, b_dir):
    in_maps = _make_in_maps(x, w_cls, b_cls, w_reg, b_reg, w_dir, b_dir)
    res = _run(in_maps, trace=False)
    return _gather(res.results)


# revision 25
# speedup vs baseline: 1.8056x; 1.1170x over previous
"""Anchor3DHead 1x1-conv head as a Trainium2 Bass/Tile kernel.

Reference computes three 1x1 convs (channels_first) over x[4, 384, 248, 216]:
  cls  = x . w_cls[384, 18] + b_cls
  reg  = x . w_reg[384, 42] + b_reg
  dir  = x . w_dir[384, 12] + b_dir

This is a per-pixel matmul over channels. We fuse the three heads into one
[384, 72] weight matrix, shard the pixels data-parallel across 8 NeuronCores
(batch 4 x H-halves 2), and run an SPMD Tile kernel: for each pixel block,
DMA x[128, 3, NBLK] in, accumulate 3 K-tiles of matmul into PSUM, add bias
on the way out of PSUM, DMA the [72, NBLK] block back.
"""

import numpy as np

import concourse.bass as bass
import concourse.mybir as mybir
from concourse import bacc
from concourse.tile import TileContext, add_dep_helper
from concourse.bass_utils import run_bass_kernel_spmd

B, C, H, W = 4, 384, 248, 216
CLS, REG, DIR = 18, 42, 12
O = CLS + REG + DIR          # 72
N_CORES = 8
HH = H // 2                  # 124 rows per core
NPIX = HH * W                # 26784 pixels per core
P = 128
KT = C // P                  # 3 contraction tiles
NSUB = 496                   # matmul free-dim tile (496*4B = 1984B, one PSUM bank)
SUBS_PER_BLK = 6
NBLK = NSUB * SUBS_PER_BLK   # 2976 pixels per superblock
BLKS = NPIX // NBLK          # 9
SIZES = ([NBLK // 6, NBLK // 3, NBLK // 2] + [NBLK] * (BLKS - 2)
         + [NBLK // 2, NBLK // 3, NBLK // 6])

_NC_CACHE = {}


def _build():
    if "nc" in _NC_CACHE:
        return _NC_CACHE["nc"]
    nc = bacc.Bacc()
    # x and w are shipped to the device as float16: halves the dominant HBM
    # read traffic vs f32, runs at the PE's full 1 cycle/row rate, and for
    # N(0,1)-scale data costs ~5e-4 relative error (f32 PSUM accumulate).
    x = nc.declare_dram_parameter("x", [C * NPIX], mybir.dt.float16, isOutput=False)
    w = nc.declare_dram_parameter("w", [C, O], mybir.dt.float16, isOutput=False)
    b = nc.declare_dram_parameter("b", [O, 1], mybir.dt.float32, isOutput=False)
    out = nc.declare_dram_parameter("out", [O, NPIX], mybir.dt.float16, isOutput=True)

    w_v = w.rearrange("(t p) m -> p t m", p=P)   # [128, 3, 72]

    with TileContext(nc) as tc:
        with (
            tc.tile_pool(name="consts", bufs=1) as consts,
            tc.tile_pool(name="xp", bufs=3) as xp,
            tc.tile_pool(name="op", bufs=2) as op,
            tc.tile_pool(name="ps", bufs=6, space="PSUM") as ps,
            tc.tile_pool(name="junkp", bufs=1, space="PSUM") as junkp,
        ):
            wt = consts.tile([P, KT, O], mybir.dt.float16)
            nc.sync.dma_start(out=wt, in_=w_v)
            bt = consts.tile([O, 1], mybir.dt.float32)
            nc.sync.dma_start(out=bt, in_=b[:, :])

            # The PE Matmult's embedded weight-load can only carry one sync
            # wait, so a matmul must never need to wait on two semaphores at
            # once. These tiny "probe" matmuls into a scratch PSUM bank absorb
            # each DMA-completion wait on the PE before the real matmuls run.
            junk = junkp.tile([1, 16], mybir.dt.float32)
            prev = nc.tensor.matmul(
                junk, wt[:, 0, 0:1], wt[:, 0, 0:16], start=True, stop=True
            )

            p0 = 0
            for blk, nblk in enumerate(SIZES):
                xt = xp.tile([P, KT, nblk], mybir.dt.float16, tag="xt")
                x_blk = x[C * p0 : C * (p0 + nblk)].rearrange(
                    "(t p n) -> p t n", t=KT, p=P
                )
                nc.sync.dma_start(out=xt, in_=x_blk)
                probe = nc.tensor.matmul(
                    junk, wt[:, 0, 0:1], xt[:, 0, 0:16], start=True, stop=True
                )
                add_dep_helper(probe.ins, prev.ins, sync=False, reason="probe order")
                ot = op.tile([O, nblk], mybir.dt.float16, tag="ot")
                for j in range(nblk // NSUB):
                    pt = ps.tile([O, NSUB], mybir.dt.float32)
                    for kk in range(KT):
                        mm = nc.tensor.matmul(
                            pt,
                            wt[:, kk, :],
                            xt[:, kk, bass.ts(j, NSUB)],
                            start=(kk == 0),
                            stop=(kk == KT - 1),
                        )
                        if j == 0 and kk == 0:
                            add_dep_helper(
                                mm.ins, probe.ins, sync=False, reason="probe first"
                            )
                    if j % 2 == 0:
                        nc.vector.tensor_scalar_add(ot[:, bass.ts(j, NSUB)], pt, bt)
                    else:
                        nc.scalar.activation(
                            ot[:, bass.ts(j, NSUB)],
                            pt,
                            mybir.ActivationFunctionType.Identity,
                            bias=bt,
                        )
                nc.gpsimd.dma_start(out=out[:, p0 : p0 + nblk], in_=ot)
                prev = probe
                p0 += nblk

    nc.finalize()
    _NC_CACHE["nc"] = nc
    return nc


def _make_in_maps(x, w_cls, b_cls, w_reg, b_reg, w_dir, b_dir):
    w = np.ascontiguousarray(
        np.concatenate([w_cls, w_reg, w_dir], axis=1), dtype=np.float32
    ).astype(np.float16)
    b = np.ascontiguousarray(
        np.concatenate([b_cls, b_reg, b_dir]), dtype=np.float32
    ).reshape(O, 1)
    x = np.asarray(x, dtype=np.float32)
    in_maps = []
    for i in range(N_CORES):
        bi, hi = divmod(i, 2)
        shard = np.ascontiguousarray(x[bi, :, hi * HH : (hi + 1) * HH, :]).reshape(
            C, NPIX
        ).astype(np.float16)
        # pack block-contiguous: for each block the [C, nblk] chunk is flat,
        # so every load DMA is one sequential DRAM sweep
        chunks = []
        p0 = 0
        for nblk in SIZES:
            chunks.append(shard[:, p0 : p0 + nblk].reshape(-1))
            p0 += nblk
        in_maps.append({"x": np.concatenate(chunks), "w": w, "b": b})
    return in_maps


def _gather(results):
    cls = np.empty((B, CLS, H, W), np.float32)
    reg = np.empty((B, REG, H, W), np.float32)
    dirp = np.empty((B, DIR, H, W), np.float32)
    for i in range(N_CORES):
        bi, hi = divmod(i, 2)
        o = np.asarray(results[i]["out"]).astype(np.float32).reshape(O, HH, W)
        cls[bi, :, hi * HH : (hi + 1) * HH, :] = o[:CLS]
        reg[bi, :, hi * HH : (hi + 1) * HH, :] = o[CLS : CLS + REG]
        dirp[bi, :, hi * HH : (hi + 1) * HH, :] = o[CLS + REG :]
    return cls, reg, dirp


def _run(in_maps, trace=False):
    nc = _build()
    return run_bass_kernel_spmd(nc, in_maps, core_ids=list(range(N_CORES)), trace=trace)


def kernel(x, w_cls, b_cls, w_reg, b_reg, w_dir, b_dir):
    in_maps = _make_in_maps(x, w_cls, b_cls, w_reg, b_reg, w_dir, b_dir)
    res = _run(in_maps, trace=False)
    return _gather(res.results)


# revision 28
# speedup vs baseline: 1.9197x; 1.0632x over previous
"""Anchor3DHead 1x1-conv head as a Trainium2 Bass/Tile kernel.

Reference computes three 1x1 convs (channels_first) over x[4, 384, 248, 216]:
  cls  = x . w_cls[384, 18] + b_cls
  reg  = x . w_reg[384, 42] + b_reg
  dir  = x . w_dir[384, 12] + b_dir

This is a per-pixel matmul over channels. We fuse the three heads into one
[384, 72] weight matrix, shard the pixels data-parallel across 8 NeuronCores
(batch 4 x H-halves 2), and run an SPMD Tile kernel: for each pixel block,
DMA x[128, 3, NBLK] in, accumulate 3 K-tiles of matmul into PSUM, add bias
on the way out of PSUM, DMA the [72, NBLK] block back.
"""

import numpy as np

import concourse.bass as bass
import concourse.mybir as mybir
from concourse import bacc
from concourse.tile import TileContext, add_dep_helper
from concourse.bass_utils import run_bass_kernel_spmd

B, C, H, W = 4, 384, 248, 216
CLS, REG, DIR = 18, 42, 12
O = CLS + REG + DIR          # 72
N_CORES = 8
HH = H // 2                  # 124 rows per core
NPIX = HH * W                # 26784 pixels per core
P = 128
KT = C // P                  # 3 contraction tiles
NSUB = 496                   # matmul free-dim tile (496*4B = 1984B, one PSUM bank)
SUBS_PER_BLK = 6
NBLK = NSUB * SUBS_PER_BLK   # 2976 pixels per superblock
BLKS = NPIX // NBLK          # 9
SIZES = ([NBLK // 6, NBLK // 3, NBLK // 2] + [NBLK] * (BLKS - 2)
         + [NBLK // 2, NBLK // 3, NBLK // 6])

_NC_CACHE = {}


def _build():
    if "nc" in _NC_CACHE:
        return _NC_CACHE["nc"]
    nc = bacc.Bacc()
    # x and w are shipped to the device as float16: halves the dominant HBM
    # read traffic vs f32, runs at the PE's full 1 cycle/row rate, and for
    # N(0,1)-scale data costs ~5e-4 relative error (f32 PSUM accumulate).
    x = nc.declare_dram_parameter("x", [C * NPIX], mybir.dt.float16, isOutput=False)
    w = nc.declare_dram_parameter("w", [C, O], mybir.dt.float16, isOutput=False)
    b = nc.declare_dram_parameter("b", [O, 1], mybir.dt.float32, isOutput=False)
    out = nc.declare_dram_parameter("out", [O, NPIX], mybir.dt.float16, isOutput=True)

    w_v = w.rearrange("(t p) m -> p t m", p=P)   # [128, 3, 72]

    with TileContext(nc) as tc:
        with (
            tc.tile_pool(name="consts", bufs=1) as consts,
            tc.tile_pool(name="xp", bufs=3) as xp,
            tc.tile_pool(name="op", bufs=2) as op,
            tc.tile_pool(name="ps", bufs=6, space="PSUM") as ps,
            tc.tile_pool(name="junkp", bufs=1, space="PSUM") as junkp,
        ):
            wt = consts.tile([P, KT, O], mybir.dt.float16)
            nc.sync.dma_start(out=wt, in_=w_v)
            bt = consts.tile([O, 1], mybir.dt.float32)
            nc.sync.dma_start(out=bt, in_=b[:, :])

            # The PE Matmult's embedded weight-load can only carry one sync
            # wait, so a matmul must never need to wait on two semaphores at
            # once. These tiny "probe" matmuls into a scratch PSUM bank absorb
            # each DMA-completion wait on the PE before the real matmuls run.
            junk = junkp.tile([1, 16], mybir.dt.float32)
            prev = nc.tensor.matmul(
                junk, wt[:, 0, 0:1], wt[:, 0, 0:16], start=True, stop=True
            )

            p0 = 0
            for blk, nblk in enumerate(SIZES):
                xt = xp.tile([P, KT, nblk], mybir.dt.float16, tag="xt")
                x_blk = x[C * p0 : C * (p0 + nblk)].rearrange(
                    "(t p n) -> p t n", t=KT, p=P
                )
                nc.sync.dma_start(out=xt, in_=x_blk)
                probe = nc.tensor.matmul(
                    junk, wt[:, 0, 0:1], xt[:, 0, 0:16], start=True, stop=True
                )
                add_dep_helper(probe.ins, prev.ins, sync=False, reason="probe order")
                ot = op.tile([O, nblk], mybir.dt.float16, tag="ot")
                nsubs = nblk // NSUB
                pts = [
                    ps.tile([O, NSUB], mybir.dt.float32, name="pt", tag="pt")
                    for _ in range(nsubs)
                ]
                # K-outer: one weight load per K-tile per block (not per group)
                for kk in range(KT):
                    for j in range(nsubs):
                        mm = nc.tensor.matmul(
                            pts[j],
                            wt[:, kk, :],
                            xt[:, kk, bass.ts(j, NSUB)],
                            start=(kk == 0),
                            stop=(kk == KT - 1),
                        )
                        if j == 0 and kk == 0:
                            add_dep_helper(
                                mm.ins, probe.ins, sync=False, reason="probe first"
                            )
                for j in range(nsubs):
                    if j % 2 == 0:
                        nc.vector.tensor_scalar_add(ot[:, bass.ts(j, NSUB)], pts[j], bt)
                    else:
                        nc.scalar.activation(
                            ot[:, bass.ts(j, NSUB)],
                            pts[j],
                            mybir.ActivationFunctionType.Identity,
                            bias=bt,
                        )
                nc.gpsimd.dma_start(out=out[:, p0 : p0 + nblk], in_=ot)
                prev = probe
                p0 += nblk

    nc.finalize()
    _NC_CACHE["nc"] = nc
    return nc


def _make_in_maps(x, w_cls, b_cls, w_reg, b_reg, w_dir, b_dir):
    w = np.ascontiguousarray(
        np.concatenate([w_cls, w_reg, w_dir], axis=1), dtype=np.float32
    ).astype(np.float16)
    b = np.ascontiguousarray(
        np.concatenate([b_cls, b_reg, b_dir]), dtype=np.float32
    ).reshape(O, 1)
    x = np.asarray(x, dtype=np.float32)
    in_maps = []
    for i in range(N_CORES):
        bi, hi = divmod(i, 2)
        shard = np.ascontiguousarray(x[bi, :, hi * HH : (hi + 1) * HH, :]).reshape(
            C, NPIX
        ).astype(np.float16)
        # pack block-contiguous: for each block the [C, nblk] chunk is flat,
        # so every load DMA is one sequential DRAM sweep
        chunks = []
        p0 = 0
        for nblk in SIZES:
            chunks.append(shard[:, p0 : p0 + nblk].reshape(-1))
            p0 += nblk
        in_maps.append({"x": np.concatenate(chunks), "w": w, "b": b})
    return in_maps


def _gather(results):
    cls = np.empty((B, CLS, H, W), np.float32)
    reg = np.empty((B, REG, H, W), np.float32)
    dirp = np.empty((B, DIR, H, W), np.float32)
    for i in range(N_CORES):
        bi, hi = divmod(i, 2)
        o = np.asarray(results[i]["out"]).astype(np.float32).reshape(O, HH, W)
        cls[bi, :, hi * HH : (hi + 1) * HH, :] = o[:CLS]
        reg[bi, :, hi * HH : (hi + 1) * HH, :] = o[CLS : CLS + REG]
        dirp[bi, :, hi * HH : (hi + 1) * HH, :] = o[CLS + REG :]
    return cls, reg, dirp


def _run(in_maps, trace=False):
    nc = _build()
    return run_bass_kernel_spmd(nc, in_maps, core_ids=list(range(N_CORES)), trace=trace)


def kernel(x, w_cls, b_cls, w_reg, b_reg, w_dir, b_dir):
    in_maps = _make_in_maps(x, w_cls, b_cls, w_reg, b_reg, w_dir, b_dir)
    res = _run(in_maps, trace=False)
    return _gather(res.results)
